# revision 10
# baseline (speedup 1.0000x reference)
"""Trainium2 Bass kernel for 0.7*BCEWithLogits + 0.3*MultiLabelMarginLoss.

Math (per row of N = B*T rows, V = 128 classes; output = mean over rows):
  bce_row = mean_n[ softplus(x_n) - x_n*t_n ]
  mlm_row = (1/V) sum_{p in pos} sum_{n in neg} relu(1 - x_p + x_n)

Key identities exploited:
  softplus(x) - x = softplus(-x), so with u = x (positives pushed to PAD=-32)
  and tab = -x_p (pads PAD), the ENTIRE bce sum is
      sum softplus(v) over every packed column v  (pads contribute ~1e-14).
  Logits are bounded (|x| < 7), so softplus(v) = ln(1 + exp(v)) directly --
  a 2-pass Exp -> Ln(e+1) chain on the ACT engine with a running accum.
  No Abs pass, no (sum+|sum|)/2 relu trick, no PE column sums, no PE at all.

  The V^2 hinge collapses to S slots per 128-row block:
      z[p,s,n] = relu(u[p,n] + tab[p,s] + 1),  accum -> one column per block.
  Pads self-kill: tab pad = -32 makes relu(u - 31) = 0.

Host does the positive extraction (sorting rows by npos, dealing them
round-robin so all 8 cores share one NEFF) and packs per core a single
bf16 array [128, 16*(128+16) + n_off]: 16 blocks side by side, each block
[u(128 cols) | tab(16 cols)], plus n_off host-precomputed ACT bias columns
(1 - x_p, pads -32) for the top slots of the heaviest blocks, which run on
the ACT engine as per-slot Relu activations (per-partition bias) to balance
DVE vs ACT. Device work:
  - 4 group DMAs (bf16, 144 KiB each), issue split Sync/GpSimd queues
  - DVE: one fused hinge op per block (the only O(S*V) work)
  - ACT: Exp + Ln(e+1) with accum = the whole BCE, + n_off Relu slots
Partial sums (per-block hinge columns + softplus accum + offload slots) DMA
out as a [128, NACC] f32 tile; the host does the final weighted reduction.
"""

import sys

sys.path.insert(0, "/opt/trn_rl_repo")

import numpy as np

import concourse.bacc as bacc
import concourse.tile as tile
from concourse import mybir
from concourse.bass_utils import run_bass_kernel_spmd

F32 = mybir.dt.float32
BF16 = mybir.dt.bfloat16
ALU = mybir.AluOpType
ACTF = mybir.ActivationFunctionType

B, T, V = 16, 1024, 128
ROWS = B * T
N_CORES = 8
RPC = ROWS // N_CORES            # 2048 rows per core
P = 128                          # rows per block
NBLK = RPC // P                  # 16 blocks
GRP = 4                          # blocks per DMA group
NGRP = NBLK // GRP
TW = 16                          # tab slots per block
CB = V + TW                      # 144 columns per block
NCOL = NBLK * CB                 # 2304 packed columns per core

PAD = -32.0
BCE_W = 0.7
MLM_W = 0.3
OFFLOAD_BUDGET = 6               # hinge slots moved from DVE to ACT
OFFLOAD_KEEP = 5                 # min slots a block keeps on DVE


def _register_ops():
    from concourse import dve_ops as dops
    from concourse.dve_spec import Spec, Src0, Src1, AluOp, relu, C0

    if hasattr(dops, "ANT_KERNEL_OPS2"):
        return dops.ANT_KERNEL_OPS2

    def _href(in0, in1, c0, c1, c2):
        i0 = in0.astype(np.float32).reshape(in0.shape[0], -1)
        i1 = in1.astype(np.float32).reshape(in0.shape[0], -1)
        b = np.maximum(i0 + i1 + c0, 0.0)
        return b, b.sum(-1, keepdims=True)

    h_spec = Spec(
        body=relu((Src0 + Src1) + C0),
        accum=AluOp.ADD, reference=_href,
    )

    ops = {}
    for name, spec in (("HINGE_SUM_ANT", h_spec),):
        opc = max(dops._SUB_OPCODE_FOR_NAME.values()) + 1
        shas = {}
        for ver in ("v3", "v4"):
            r = dops.DveOpSpec(
                name=name, opcode=opc,
                uops=dops.lower(spec, ver=ver), rd1_en=dops.has_src1(spec),
            )
            shas[ver] = r.sha(ver)
        op = dops.DveOp(name, spec, subdim=False, uops_sha=shas)
        dops.OPS.append(op)
        dops.CUSTOM_DVE_SPECS[name] = spec
        dops._SUB_OPCODE_FOR_NAME[name] = opc
        ops[name] = op
    dops.ANT_KERNEL_OPS2 = ops
    return ops


_OPS = _register_ops()
HINGE = _OPS["HINGE_SUM_ANT"]


def _act_set_id(nc):
    from concourse.hw_specs import get_activation_tables

    return list(get_activation_tables(nc.m.arch)).index("natural_log_exp_and_others")


def _offload_for(schedule):
    """Per-block count of top hinge slots moved to the ACT engine."""
    budget = OFFLOAD_BUDGET
    off = [0] * NBLK
    for b in sorted(range(NBLK), key=lambda b: -schedule[b]):
        if budget <= 0:
            break
        a = min(budget, max(0, schedule[b] - OFFLOAD_KEEP))
        off[b] = a
        budget -= a
    return tuple(off)


CHUNKS = ((0, 2), (2, 8), (8, 13), (13, 16))   # block ranges per DMA chunk


def build_nc(schedule, offload):
    """schedule: per-block hinge-slot counts (>=1). offload: per-block count
    of top slots handled by ACT instead of DVE."""
    n_off = sum(offload)
    ntot = NCOL + n_off
    sdve = [schedule[b] - offload[b] for b in range(NBLK)]

    # Per chunk choose the formulation with fewer DVE instructions:
    # per-block [P,S,V] ops, or (if S ascending) per-slot ops over the
    # suffix of blocks with S > s (consecutive in memory, stride CB).
    plans = []
    n_dve = 0
    for b0, b1 in CHUNKS:
        ss = sdve[b0:b1]
        nb, smax = b1 - b0, max(ss)
        asc = all(ss[i] <= ss[i + 1] for i in range(len(ss) - 1))
        if asc and smax < nb:
            plans.append(("slot", b0, b1))
            n_dve += smax
        else:
            plans.append(("block", b0, b1))
            n_dve += sum(1 for s in ss if s > 0)
    nacc = n_dve + n_off + 1                    # softplus accum is last

    nc = bacc.Bacc("TRN2", target_bir_lowering=False, debug=False)
    xp_dram = nc.dram_tensor("xp", [P, ntot], BF16, kind="ExternalInput")
    out_dram = nc.dram_tensor("out", [P, nacc], F32, kind="ExternalOutput")
    xp_ap = xp_dram.ap()

    with tile.TileContext(nc) as tc:
        with (
            tc.tile_pool(name="inp", bufs=1) as ipool,
            tc.tile_pool(name="ex", bufs=1) as epool,
            tc.tile_pool(name="zp", bufs=4) as zpool,
            tc.tile_pool(name="za", bufs=2) as za_pool,
            tc.tile_pool(name="accs", bufs=1) as apool,
        ):
            inp = ipool.tile([P, ntot], BF16, tag="inp")
            acc = apool.tile([P, nacc], F32, tag="acc")

            # Input DMA in 4 chunks, issued from three engines so descriptor
            # generation overlaps (~650ns per issue); small first chunk gets
            # the DVE started earliest. Last chunk carries the ACT bias cols.
            bounds = [CHUNKS[0][0] * CB, CHUNKS[1][0] * CB,
                      CHUNKS[2][0] * CB, CHUNKS[3][0] * CB, ntot]
            issuers = [nc.sync, nc.scalar, nc.sync, nc.gpsimd]
            for g in range(4):
                issuers[g].dma_start(
                    inp[:, bounds[g] : bounds[g + 1]],
                    xp_ap[:, bounds[g] : bounds[g + 1]],
                )

            # Pin the ln+exp table while the Scalar engine waits for data
            # (keeps the implicit per-activation load off the critical path).
            nc.scalar.add_instruction(
                mybir.InstLoadActFuncSet(
                    name=nc.get_next_instruction_name(), ins=[], outs=[],
                    act_func_set_id=_act_set_id(nc),
                )
            )

            # DVE: fused hinge ops, one accum column each
            ai = 0
            for kind, b0, b1 in plans:
                if kind == "block":
                    for b in range(b0, b1):
                        S = sdve[b]
                        if S <= 0:
                            continue
                        u = inp[:, b * CB : b * CB + V]
                        t = inp[:, b * CB + V : b * CB + V + S]
                        zr = zpool.tile([P, S * V], BF16, tag="zr")
                        zv = zr[:].rearrange("p (s n) -> p s n", s=S)
                        nc.vector._custom_dve(
                            HINGE,
                            out=zv,
                            in0=u.unsqueeze(1).broadcast_to([P, S, V]),
                            in1=t.unsqueeze(2).broadcast_to([P, S, V]),
                            s0=1.0,
                            accum_out=acc[:, ai : ai + 1],
                        )
                        ai += 1
                else:
                    smax = max(sdve[b0:b1])
                    for s in range(smax):
                        f = next(b for b in range(b0, b1) if sdve[b] > s)
                        nb = b1 - f
                        cv = inp[:, f * CB : b1 * CB].rearrange(
                            "p (b c) -> p b c", c=CB
                        )
                        u = cv[:, :, 0:V]                       # [P, nb, V]
                        t = cv[:, :, V + s : V + s + 1]         # [P, nb, 1]
                        zr = zpool.tile([P, nb * V], BF16, tag="zr")
                        zv = zr[:].rearrange("p (b n) -> p b n", b=nb)
                        nc.vector._custom_dve(
                            HINGE,
                            out=zv,
                            in0=u,
                            in1=t.broadcast_to([P, nb, V]),
                            s0=1.0,
                            accum_out=acc[:, ai : ai + 1],
                        )
                        ai += 1

            # ACT: softplus = Ln(Exp(v) + 1); Exp split in two so it can
            # start before the last chunk lands. Accum on the Ln pass only.
            mid = CHUNKS[1][1] * CB
            e = epool.tile([P, NCOL], F32, tag="e")
            nc.scalar.activation(e[:, :mid], inp[:, :mid], ACTF.Exp,
                                 bias=0.0, scale=1.0)
            nc.scalar.activation(e[:, mid:NCOL], inp[:, mid:NCOL], ACTF.Exp,
                                 bias=0.0, scale=1.0)
            l = epool.tile([P, NCOL], BF16, tag="l")
            nc.scalar.activation(
                l[:], e[:], ACTF.Ln, bias=1.0, scale=1.0,
                accum_out=acc[:, nacc - 1 : nacc],
            )

            # ACT: offloaded hinge slots, one Relu per slot, per-row bias
            # columns (1 - x_p) prepped by the host at inp[:, NCOL:].
            i = 0
            for b in range(NBLK):
                for _ in range(offload[b]):
                    u = inp[:, b * CB : b * CB + V]
                    za = za_pool.tile([P, V], BF16, tag="za")
                    nc.scalar.activation(
                        za[:], u, ACTF.Relu,
                        bias=inp[:, NCOL + i : NCOL + i + 1], scale=1.0,
                        accum_out=acc[:, ai : ai + 1],
                    )
                    ai += 1
                    i += 1

            nc.sync.dma_start(out_dram.ap()[:, :], acc[:])

    nc.compile()
    return nc


_NC_CACHE = {}


def _get_nc(schedule, offload):
    key = (schedule, offload)
    if key not in _NC_CACHE:
        _NC_CACHE[key] = build_nc(schedule, offload)
    return _NC_CACHE[key]


def _pack(x, t):
    """Sort rows by npos, deal round-robin to cores, extract positives.
    Returns (schedule, offload, [per-core [P, NCOL+n_off] bf16 arrays])."""
    import ml_dtypes

    pos = t > 0.5
    npos = pos.sum(axis=1)
    order = np.argsort(npos, kind="stable")
    xs = x[order]
    ps = pos[order]
    ns = npos[order]
    schedule = tuple(
        max(1, int(ns[(b + 1) * (N_CORES * P) - 1])) for b in range(NBLK)
    )
    assert max(schedule) <= TW, f"npos {max(schedule)} exceeds tab width {TW}"
    offload = _offload_for(schedule)

    u = np.where(ps, np.float32(PAD), xs)
    idx = np.argsort(~ps, axis=1, kind="stable")[:, :TW]
    pv = np.take_along_axis(xs, idx, axis=1)
    valid = np.arange(TW)[None, :] < ns[:, None]
    tab = np.where(valid, -pv, np.float32(PAD))
    full = np.concatenate([u, tab], axis=1)            # [ROWS, CB] f32

    # ACT bias columns: per offloaded (block, slot) one column of 1 - x_p
    # ( = 1 + tab value), pads PAD so relu(u + PAD) == 0.
    bias_blocks = []
    for b in range(NBLK):
        a = offload[b]
        if a:
            s0 = schedule[b] - a
            bias_blocks.append((b, s0, a))

    shards = []
    n_off = sum(offload)
    for c in range(N_CORES):
        s = full[c::N_CORES]                           # [RPC, CB]
        sb = s.reshape(NBLK, P, CB).transpose(1, 0, 2).reshape(P, NCOL)
        cols = [sb]
        for b, s0, a in bias_blocks:
            tb = s[b * P : (b + 1) * P, V + s0 : V + s0 + a]   # tab slice
            bc = np.where(tb > PAD / 2, tb + np.float32(1.0), np.float32(PAD))
            cols.append(bc)
        arr = np.concatenate(cols, axis=1) if n_off else sb
        shards.append(np.ascontiguousarray(arr.astype(ml_dtypes.bfloat16)))
    return schedule, offload, shards


def _combine(accs):
    """accs: list of [P, nacc] f32 per core -> scalar loss."""
    h = 0.0
    sp = 0.0
    for a in accs:
        a = np.asarray(a, np.float64)
        h += a[:, :-1].sum()
        sp += a[:, -1].sum()
    return np.float32((BCE_W * sp + MLM_W * h) / (V * ROWS))


def kernel(logits: np.ndarray, targets: np.ndarray) -> np.ndarray:
    x = np.asarray(logits, dtype=np.float32).reshape(ROWS, V)
    t = np.asarray(targets, dtype=np.float32).reshape(ROWS, V)
    schedule, offload, shards = _pack(x, t)
    nc = _get_nc(schedule, offload)
    in_maps = [{"xp": shards[c]} for c in range(N_CORES)]
    res = run_bass_kernel_spmd(nc, in_maps, list(range(N_CORES)))
    return _combine([res.results[c]["out"] for c in range(N_CORES)])


# revision 11
# speedup vs baseline: 1.1232x; 1.1232x over previous
"""Trainium2 Bass kernel for 0.7*BCEWithLogits + 0.3*MultiLabelMarginLoss.

Math (per row of N = B*T rows, V = 128 classes; output = mean over rows):
  bce_row = mean_n[ softplus(x_n) - x_n*t_n ]
  mlm_row = (1/V) sum_{p in pos} sum_{n in neg} relu(1 - x_p + x_n)

Key identities exploited:
  softplus(x) - x = softplus(-x), so with u = x (positives pushed to PAD=-32)
  and tab = -x_p (pads PAD), the ENTIRE bce sum is
      sum softplus(v) over every packed column v  (pads contribute ~1e-14).
  Logits are bounded (|x| < 7), so softplus(v) = ln(1 + exp(v)) directly --
  a 2-pass Exp -> Ln(e+1) chain on the ACT engine with a running accum.
  No Abs pass, no (sum+|sum|)/2 relu trick, no PE column sums, no PE at all.

  The V^2 hinge collapses to S slots per 128-row block:
      z[p,s,n] = relu(u[p,n] + tab[p,s] + 1),  accum -> one column per block.
  Pads self-kill: tab pad = -32 makes relu(u - 31) = 0.

Host does the positive extraction (sorting rows by npos, dealing them
round-robin so all 8 cores share one NEFF) and packs per core a single
bf16 array [128, 16*(128+16) + n_off]: 16 blocks side by side, each block
[u(128 cols) | tab(16 cols)], plus n_off host-precomputed ACT bias columns
(1 - x_p, pads -32) for the top slots of the heaviest blocks, which run on
the ACT engine as per-slot Relu activations (per-partition bias) to balance
DVE vs ACT. Device work:
  - 4 group DMAs (bf16, 144 KiB each), issue split Sync/GpSimd queues
  - DVE: one fused hinge op per block (the only O(S*V) work)
  - ACT: Exp + Ln(e+1) with accum = the whole BCE, + n_off Relu slots
Partial sums (per-block hinge columns + softplus accum + offload slots) DMA
out as a [128, NACC] f32 tile; the host does the final weighted reduction.
"""

import sys

sys.path.insert(0, "/opt/trn_rl_repo")

import numpy as np

import concourse.bacc as bacc
import concourse.tile as tile
from concourse import mybir
from concourse.bass_utils import run_bass_kernel_spmd

F32 = mybir.dt.float32
BF16 = mybir.dt.bfloat16
ALU = mybir.AluOpType
ACTF = mybir.ActivationFunctionType

B, T, V = 16, 1024, 128
ROWS = B * T
N_CORES = 8
RPC = ROWS // N_CORES            # 2048 rows per core
P = 128                          # rows per block
NBLK = RPC // P                  # 16 blocks
GRP = 4                          # blocks per DMA group
NGRP = NBLK // GRP
TW = 16                          # tab slots per block
CB = V + TW                      # 144 columns per block
NCOL = NBLK * CB                 # 2304 packed columns per core

PAD = -32.0
BCE_W = 0.7
MLM_W = 0.3
OFFLOAD_BUDGET = 4               # hinge slots moved from DVE to ACT
OFFLOAD_KEEP = 5                 # min slots a block keeps on DVE


def _register_ops():
    from concourse import dve_ops as dops
    from concourse.dve_spec import Spec, Src0, Src1, AluOp, relu, C0

    if hasattr(dops, "ANT_KERNEL_OPS2"):
        return dops.ANT_KERNEL_OPS2

    def _href(in0, in1, c0, c1, c2):
        i0 = in0.astype(np.float32).reshape(in0.shape[0], -1)
        i1 = in1.astype(np.float32).reshape(in0.shape[0], -1)
        b = np.maximum(i0 + i1 + c0, 0.0)
        return b, b.sum(-1, keepdims=True)

    h_spec = Spec(
        body=relu((Src0 + Src1) + C0),
        accum=AluOp.ADD, reference=_href,
    )

    ops = {}
    for name, spec in (("HINGE_SUM_ANT", h_spec),):
        opc = max(dops._SUB_OPCODE_FOR_NAME.values()) + 1
        shas = {}
        for ver in ("v3", "v4"):
            r = dops.DveOpSpec(
                name=name, opcode=opc,
                uops=dops.lower(spec, ver=ver), rd1_en=dops.has_src1(spec),
            )
            shas[ver] = r.sha(ver)
        op = dops.DveOp(name, spec, subdim=False, uops_sha=shas)
        dops.OPS.append(op)
        dops.CUSTOM_DVE_SPECS[name] = spec
        dops._SUB_OPCODE_FOR_NAME[name] = opc
        ops[name] = op
    dops.ANT_KERNEL_OPS2 = ops
    return ops


_OPS = _register_ops()
HINGE = _OPS["HINGE_SUM_ANT"]


def _act_set_id(nc):
    from concourse.hw_specs import get_activation_tables

    return list(get_activation_tables(nc.m.arch)).index("natural_log_exp_and_others")


def _offload_for(schedule):
    """Per-block count of top hinge slots moved to the ACT engine."""
    budget = OFFLOAD_BUDGET
    off = [0] * NBLK
    for b in sorted(range(NBLK), key=lambda b: -schedule[b]):
        if budget <= 0:
            break
        a = min(budget, max(0, schedule[b] - OFFLOAD_KEEP))
        off[b] = a
        budget -= a
    return tuple(off)


CHUNKS = ((0, 2), (2, 8), (8, 13), (13, 16))   # block ranges per DMA chunk


def build_nc(schedule, offload):
    """schedule: per-block hinge-slot counts (>=1). offload: per-block count
    of top slots handled by ACT instead of DVE."""
    n_off = sum(offload)
    ntot = NCOL + n_off
    sdve = [schedule[b] - offload[b] for b in range(NBLK)]

    # Per chunk choose the formulation with fewer DVE instructions:
    # per-block [P,S,V] ops, or (if S ascending) per-slot ops over the
    # suffix of blocks with S > s (consecutive in memory, stride CB).
    plans = []
    n_dve = 0
    for b0, b1 in CHUNKS:
        ss = sdve[b0:b1]
        nb, smax = b1 - b0, max(ss)
        asc = all(ss[i] <= ss[i + 1] for i in range(len(ss) - 1))
        if asc and smax < nb:
            plans.append(("slot", b0, b1))
            n_dve += smax
        else:
            plans.append(("block", b0, b1))
            n_dve += sum(1 for s in ss if s > 0)
    nacc = n_dve + n_off + 1                    # softplus accum is last

    nc = bacc.Bacc("TRN2", target_bir_lowering=False, debug=False)
    xp_dram = nc.dram_tensor("xp", [P, ntot], BF16, kind="ExternalInput")
    out_dram = nc.dram_tensor("out", [P, nacc], F32, kind="ExternalOutput")
    xp_ap = xp_dram.ap()

    with tile.TileContext(nc) as tc:
        with (
            tc.tile_pool(name="inp", bufs=1) as ipool,
            tc.tile_pool(name="ex", bufs=1) as epool,
            tc.tile_pool(name="zp", bufs=4) as zpool,
            tc.tile_pool(name="za", bufs=2) as za_pool,
            tc.tile_pool(name="accs", bufs=1) as apool,
        ):
            inp = ipool.tile([P, ntot], BF16, tag="inp")
            acc = apool.tile([P, nacc], F32, tag="acc")

            # Input DMA in 4 chunks, issued from three engines so descriptor
            # generation overlaps (~650ns per issue); small first chunk gets
            # the DVE started earliest. Last chunk carries the ACT bias cols.
            bounds = [CHUNKS[0][0] * CB, CHUNKS[1][0] * CB,
                      CHUNKS[2][0] * CB, CHUNKS[3][0] * CB, ntot]
            issuers = [nc.sync, nc.scalar, nc.sync, nc.gpsimd]
            for g in range(4):
                issuers[g].dma_start(
                    inp[:, bounds[g] : bounds[g + 1]],
                    xp_ap[:, bounds[g] : bounds[g + 1]],
                )

            # Pin the ln+exp table while the Scalar engine waits for data
            # (keeps the implicit per-activation load off the critical path).
            nc.scalar.add_instruction(
                mybir.InstLoadActFuncSet(
                    name=nc.get_next_instruction_name(), ins=[], outs=[],
                    act_func_set_id=_act_set_id(nc),
                )
            )

            # DVE: fused hinge ops, one accum column each
            ai = 0
            for kind, b0, b1 in plans:
                if kind == "block":
                    for b in range(b0, b1):
                        S = sdve[b]
                        if S <= 0:
                            continue
                        u = inp[:, b * CB : b * CB + V]
                        t = inp[:, b * CB + V : b * CB + V + S]
                        zr = zpool.tile([P, S * V], BF16, tag="zr")
                        zv = zr[:].rearrange("p (s n) -> p s n", s=S)
                        nc.vector._custom_dve(
                            HINGE,
                            out=zv,
                            in0=u.unsqueeze(1).broadcast_to([P, S, V]),
                            in1=t.unsqueeze(2).broadcast_to([P, S, V]),
                            s0=1.0,
                            accum_out=acc[:, ai : ai + 1],
                        )
                        ai += 1
                else:
                    smax = max(sdve[b0:b1])
                    for s in range(smax):
                        f = next(b for b in range(b0, b1) if sdve[b] > s)
                        nb = b1 - f
                        cv = inp[:, f * CB : b1 * CB].rearrange(
                            "p (b c) -> p b c", c=CB
                        )
                        u = cv[:, :, 0:V]                       # [P, nb, V]
                        t = cv[:, :, V + s : V + s + 1]         # [P, nb, 1]
                        zr = zpool.tile([P, nb * V], BF16, tag="zr")
                        zv = zr[:].rearrange("p (b n) -> p b n", b=nb)
                        nc.vector._custom_dve(
                            HINGE,
                            out=zv,
                            in0=u,
                            in1=t.broadcast_to([P, nb, V]),
                            s0=1.0,
                            accum_out=acc[:, ai : ai + 1],
                        )
                        ai += 1

            # ACT: softplus = Ln(Exp(v) + 1); Exp split in two so it can
            # start before the last chunk lands. Accum on the Ln pass only.
            mid = CHUNKS[1][1] * CB
            e = epool.tile([P, NCOL], F32, tag="e")
            nc.scalar.activation(e[:, :mid], inp[:, :mid], ACTF.Exp,
                                 bias=0.0, scale=1.0)
            nc.scalar.activation(e[:, mid:NCOL], inp[:, mid:NCOL], ACTF.Exp,
                                 bias=0.0, scale=1.0)
            l = epool.tile([P, NCOL], BF16, tag="l")
            nc.scalar.activation(
                l[:], e[:], ACTF.Ln, bias=1.0, scale=1.0,
                accum_out=acc[:, nacc - 1 : nacc],
            )

            # ACT: offloaded hinge slots, one Relu per slot, per-row bias
            # columns (1 - x_p) prepped by the host at inp[:, NCOL:].
            i = 0
            for b in range(NBLK):
                for _ in range(offload[b]):
                    u = inp[:, b * CB : b * CB + V]
                    za = za_pool.tile([P, V], BF16, tag="za")
                    nc.scalar.activation(
                        za[:], u, ACTF.Relu,
                        bias=inp[:, NCOL + i : NCOL + i + 1], scale=1.0,
                        accum_out=acc[:, ai : ai + 1],
                    )
                    ai += 1
                    i += 1

            nc.sync.dma_start(out_dram.ap()[:, :], acc[:])

    nc.compile()
    return nc


_NC_CACHE = {}


def _get_nc(schedule, offload):
    key = (schedule, offload)
    if key not in _NC_CACHE:
        _NC_CACHE[key] = build_nc(schedule, offload)
    return _NC_CACHE[key]


def _pack(x, t):
    """Sort rows by npos, deal round-robin to cores, extract positives.
    Returns (schedule, offload, [per-core [P, NCOL+n_off] bf16 arrays])."""
    import ml_dtypes

    pos = t > 0.5
    npos = pos.sum(axis=1)
    order = np.argsort(npos, kind="stable")
    xs = x[order]
    ps = pos[order]
    ns = npos[order]
    schedule = tuple(
        max(1, int(ns[(b + 1) * (N_CORES * P) - 1])) for b in range(NBLK)
    )
    assert max(schedule) <= TW, f"npos {max(schedule)} exceeds tab width {TW}"
    offload = _offload_for(schedule)

    u = np.where(ps, np.float32(PAD), xs)
    idx = np.argsort(~ps, axis=1, kind="stable")[:, :TW]
    pv = np.take_along_axis(xs, idx, axis=1)
    valid = np.arange(TW)[None, :] < ns[:, None]
    tab = np.where(valid, -pv, np.float32(PAD))
    full = np.concatenate([u, tab], axis=1)            # [ROWS, CB] f32

    # ACT bias columns: per offloaded (block, slot) one column of 1 - x_p
    # ( = 1 + tab value), pads PAD so relu(u + PAD) == 0.
    bias_blocks = []
    for b in range(NBLK):
        a = offload[b]
        if a:
            s0 = schedule[b] - a
            bias_blocks.append((b, s0, a))

    shards = []
    n_off = sum(offload)
    for c in range(N_CORES):
        s = full[c::N_CORES]                           # [RPC, CB]
        sb = s.reshape(NBLK, P, CB).transpose(1, 0, 2).reshape(P, NCOL)
        cols = [sb]
        for b, s0, a in bias_blocks:
            tb = s[b * P : (b + 1) * P, V + s0 : V + s0 + a]   # tab slice
            bc = np.where(tb > PAD / 2, tb + np.float32(1.0), np.float32(PAD))
            cols.append(bc)
        arr = np.concatenate(cols, axis=1) if n_off else sb
        shards.append(np.ascontiguousarray(arr.astype(ml_dtypes.bfloat16)))
    return schedule, offload, shards


def _combine(accs):
    """accs: list of [P, nacc] f32 per core -> scalar loss."""
    h = 0.0
    sp = 0.0
    for a in accs:
        a = np.asarray(a, np.float64)
        h += a[:, :-1].sum()
        sp += a[:, -1].sum()
    return np.float32((BCE_W * sp + MLM_W * h) / (V * ROWS))


def kernel(logits: np.ndarray, targets: np.ndarray) -> np.ndarray:
    x = np.asarray(logits, dtype=np.float32).reshape(ROWS, V)
    t = np.asarray(targets, dtype=np.float32).reshape(ROWS, V)
    schedule, offload, shards = _pack(x, t)
    nc = _get_nc(schedule, offload)
    in_maps = [{"xp": shards[c]} for c in range(N_CORES)]
    res = run_bass_kernel_spmd(nc, in_maps, list(range(N_CORES)))
    return _combine([res.results[c]["out"] for c in range(N_CORES)])
